# revision 1
# baseline (speedup 1.0000x reference)
"""Multi-head attention (B=4, S=2048, D=768, H=12) on 8 Trainium2 cores.

Sharding: the 48 (batch, head) pairs are data-parallel; each core gets 6.
Per head on one core (all matmuls bf16, fp32 PSUM accumulation):
  QT/KT [128, S]  : col/row-duplicated so score matmuls can row-tile
                    (two K=64 matmuls run concurrently in the PE array)
  V     [S, 64+1] : ones column appended -> AV matmul also produces the
                    softmax denominator (normalization folded to the end)
  scoresT [k, q]  : per 128-row k-chunk, [128, 1024] PSUM tiles
  P = exp(s/8)    : ACT engine, PSUM -> SBUF bf16 (the throughput wall)
  out^T [65, 512] = sum_k V_aug^T P; row 64 = softmax denominator.
                    Denominator rows bounce through DRAM and return as one
                    [128, 16] tile for a single cheap reciprocal; a 0-stride
                    DMA re-read broadcasts each reciprocal row across
                    partitions for one DVE mul per 512-chunk. Output lands
                    in [e, q] layout; the host gather transposes it back.

Scheduling: ACT is the bottleneck engine, so score-pair emission (the
only producer of ACT work) is interleaved 1:N with "filler" PE work
(AV matmuls of the previous q-block, QKV of the next head) via a FIFO
of generators, keeping both engines dense.
"""

import sys
from collections import deque

for _p in ("/opt/trn_rl_repo",):
    if _p not in sys.path:
        sys.path.insert(0, _p)

import numpy as np

B, S, D, H = 4, 2048, 768, 12
DH = 64
NCORES = 8
HPC = (B * H) // NCORES  # 6 heads per core
SCALE = 1.0 / 8.0
NKC = S // 128  # 16 k-chunks
NQB = 2  # q blocks of 1024
QB = S // NQB
PUMPS_PER_PAIR = 5


def _split_multi_waits(nc):
    """This walrus build rejects >1 sync wait per instruction. Insert
    single-wait NoOps (same engine, so same instruction stream) ahead of
    any instruction carrying several waits."""
    import bass_rust
    import concourse.mybir as mybir

    n_split = 0
    for f in nc.m.functions:
        for bb in f.blocks:
            out = []
            dirty = False
            for inst in bb.instructions:
                si = inst.sync_info
                if si is not None and len(si.on_wait) > 1:
                    waits = list(si.on_wait)
                    for j, w in enumerate(waits[:-1]):
                        nop = mybir.InstNoOp(name=f"{inst.name}-w{j}", ins=[], outs=[])
                        nop.engine = inst.engine
                        nop.sync_info = bass_rust.SyncInfo(on_wait=[w], on_update=[])
                        out.append(nop)
                    si.on_wait = waits[-1:]
                    dirty = True
                    n_split += 1
                out.append(inst)
            if dirty:
                bb.instructions = out
    return n_split


_BUILT = None


def build():
    global _BUILT
    if _BUILT is not None:
        return _BUILT
    import concourse.bass as bass
    import concourse.mybir as mybir
    import concourse.tile as tile

    F32 = mybir.dt.float32
    BF = mybir.dt.bfloat16
    AF = mybir.ActivationFunctionType

    nc = bass.Bass()
    xTd = nc.dram_tensor("xT", [HPC, 128, S], BF, kind="ExternalInput")
    wqkd = nc.dram_tensor("wqk", [HPC, 64, 2, 64], BF, kind="ExternalInput")
    wvTd = nc.dram_tensor("wvT", [HPC, 128, 64], BF, kind="ExternalInput")
    bqkd = nc.dram_tensor("bqk", [HPC, 128, 2], F32, kind="ExternalInput")
    bvd = nc.dram_tensor("bv", [HPC, 1, 64], F32, kind="ExternalInput")
    outd = nc.dram_tensor("out", [HPC, 64, S], F32, kind="ExternalOutput")
    dnd = nc.dram_tensor("dnd", [HPC, 4, 512], F32)  # denominator bounce
    rcd = nc.dram_tensor("rcd", [HPC, 4, 512], F32)  # reciprocal bounce

    with tile.TileContext(nc) as tc:
        with (
            tc.tile_pool(name="const", bufs=1) as cpool,
            tc.tile_pool(name="x", bufs=2) as xpool,
            tc.tile_pool(name="w", bufs=2) as wpool,
            tc.tile_pool(name="qk", bufs=2) as qkpool,
            tc.tile_pool(name="v", bufs=2) as vpool,
            tc.tile_pool(name="pt", bufs=2 * NKC * NQB) as ptpool,
            tc.tile_pool(name="ot", bufs=9) as otpool,
            tc.tile_pool(name="r", bufs=3) as rpool,
            tc.tile_pool(name="sp", bufs=3, space="PSUM") as sppool,
            tc.tile_pool(name="avp", bufs=2, space="PSUM") as avpool,
        ):
            ones1 = cpool.tile([1, 128], F32, tag="ones1")
            nc.vector.memset(ones1[:], 1.0)

            state = {}

            def qkv_steps(i):
                x_t = xpool.tile([128, S], BF, tag="x", name=f"x{i}")
                nc.gpsimd.dma_start(x_t[:], xTd[i])
                w_t = wpool.tile([64, 2, 64], BF, tag="wqk", name=f"wqk{i}")
                nc.gpsimd.dma_start(w_t[:], wqkd[i])
                wv_t = wpool.tile([128, 64], BF, tag="wv", name=f"wv{i}")
                nc.gpsimd.dma_start(wv_t[:], wvTd[i])
                b_t = wpool.tile([128, 2], F32, tag="bqk", name=f"bqk{i}")
                nc.gpsimd.dma_start(b_t[:], bqkd[i])
                bv_t = wpool.tile([1, 64], F32, tag="bv", name=f"bv{i}")
                nc.gpsimd.dma_start(bv_t[:], bvd[i])

                # bias(V) broadcast along partitions via K=1 outer product
                bv_ps = sppool.tile([128, 64], F32, tag="sp", name=f"bvp{i}")
                nc.tensor.matmul(bv_ps[:], ones1[:], bv_t[:])
                bv_sb = wpool.tile([128, 64], F32, tag="bvsb", name=f"bvs{i}")
                nc.vector.tensor_copy(bv_sb[:], bv_ps[:])
                yield

                # QT/KT duplicated into both partition halves (col-tiled pair)
                qt = qkpool.tile([128, S], BF, tag="qt", name=f"qt{i}")
                kt = qkpool.tile([128, S], BF, tag="kt", name=f"kt{i}")
                for dst, wsel, bsel in ((qt, 0, 0), (kt, 1, 1)):
                    for qm in range(4):
                        ps = sppool.tile(
                            [128, 512], F32, tag="sp", name=f"qk{i}_{wsel}_{qm}"
                        )
                        rhs = x_t[0:64, qm * 512 : (qm + 1) * 512]
                        nc.tensor.matmul(
                            ps[0:64, :], w_t[:, wsel, :], rhs, tile_position=(0, 0)
                        )
                        nc.tensor.matmul(
                            ps[64:128, :], w_t[:, wsel, :], rhs, tile_position=(0, 64)
                        )
                        nc.vector.tensor_scalar_add(
                            dst[:, qm * 512 : (qm + 1) * 512],
                            ps[:],
                            b_t[:, bsel : bsel + 1],
                        )
                        yield

                # V with ones column (denominator trick), row-tiled pairs
                v_sb = vpool.tile([128, NKC, 65], BF, tag="v", name=f"v{i}")
                nc.vector.memset(v_sb[:, :, 64:65], 1.0)
                for p in range(8):
                    psA = sppool.tile([128, 64], F32, tag="sp", name=f"vA{i}_{p}")
                    psB = sppool.tile([128, 64], F32, tag="sp", name=f"vB{i}_{p}")
                    nc.tensor.matmul(
                        psA[:],
                        x_t[0:64, p * 128 : (p + 1) * 128],
                        wv_t[0:64, :],
                        tile_position=(0, 0),
                    )
                    nc.tensor.matmul(
                        psB[:],
                        x_t[64:128, (p + 8) * 128 : (p + 9) * 128],
                        wv_t[64:128, :],
                        tile_position=(64, 0),
                    )
                    nc.vector.tensor_add(v_sb[:, p, 0:64], psA[:], bv_sb[:])
                    nc.vector.tensor_add(v_sb[:, p + 8, 0:64], psB[:], bv_sb[:])
                    if p % 2 == 1:
                        yield
                state[i] = {"qt": qt, "kt": kt, "v": v_sb, "pt": {}}

            def sc_pair(i, jb, kc):
                """One kc-pair of row-tiled score matmuls + their exps."""
                qt, kt = state[i]["qt"], state[i]["kt"]
                pt = state[i]["pt"].setdefault(jb, [None] * NKC)
                tA = sppool.tile([128, QB], F32, tag="sp", name=f"sA{i}_{jb}_{kc}")
                tB = sppool.tile([128, QB], F32, tag="sp", name=f"sB{i}_{jb}_{kc}")
                lA = kt[0:64, kc * 128 : (kc + 1) * 128]
                lB = kt[64:128, (kc + 8) * 128 : (kc + 9) * 128]
                # same-lhsT matmuls adjacent so ldw-opt can skip the reload
                for qm in range(QB // 512):
                    q0 = jb * QB + qm * 512
                    sl = slice(qm * 512, (qm + 1) * 512)
                    nc.tensor.matmul(
                        tA[:, sl], lA, qt[0:64, q0 : q0 + 512], tile_position=(0, 0)
                    )
                for qm in range(QB // 512):
                    q0 = jb * QB + qm * 512
                    sl = slice(qm * 512, (qm + 1) * 512)
                    nc.tensor.matmul(
                        tB[:, sl], lB, qt[64:128, q0 : q0 + 512], tile_position=(64, 0)
                    )
                pA = ptpool.tile([128, QB], BF, tag="pt", name=f"pA{i}_{jb}_{kc}")
                pB = ptpool.tile([128, QB], BF, tag="pt", name=f"pB{i}_{jb}_{kc}")
                nc.scalar.activation(pA[:], tA[:], AF.Exp, scale=SCALE)
                nc.scalar.activation(pB[:], tB[:], AF.Exp, scale=SCALE)
                pt[kc] = pA
                pt[kc + 8] = pB

            def av_steps(i, jb):
                """Generator: AV matmuls in groups of 4; numerators parked in
                SBUF, denominator rows bounced to DRAM. After the last
                q-chunk of the head: one [128,16] reciprocal, bounce back,
                broadcast-read per chunk, multiply, store."""
                v_sb = state[i]["v"]
                pt = state[i]["pt"].pop(jb)
                ots_list = state[i].setdefault("ots", [])
                for qm in range(QB // 512):
                    av = avpool.tile([128, 512], F32, tag="av", name=f"av{i}_{jb}_{qm}")
                    for kc in range(NKC):
                        nc.tensor.matmul(
                            av[0:65, :],
                            v_sb[:, kc, :],
                            pt[kc][:, qm * 512 : (qm + 1) * 512],
                            start=(kc == 0),
                            stop=(kc == NKC - 1),
                        )
                        if kc % 4 == 3:
                            yield
                    g = jb * (QB // 512) + qm
                    ots = otpool.tile([65, 512], F32, tag="ot", name=f"ot{i}_{jb}_{qm}")
                    nc.vector.tensor_copy(ots[:], av[0:65, :])
                    nc.sync.dma_start(dnd[i, g : g + 1], ots[64:65, :])
                    ots_list.append(ots)
                    yield
                if jb == NQB - 1:
                    den4 = rpool.tile([128, 16], F32, tag="r", name=f"dn{i}")
                    nc.sync.dma_start(
                        den4[:], dnd[i].rearrange("a c -> (a c)").rearrange("(p c) -> p c", c=16)
                    )
                    r4 = rpool.tile([128, 16], F32, tag="r", name=f"rc{i}")
                    nc.vector.reciprocal(r4[:], den4[:])
                    nc.sync.dma_start(
                        rcd[i].rearrange("a c -> (a c)").rearrange("(p c) -> p c", c=16),
                        r4[:],
                    )
                    yield
                    for g, ots in enumerate(ots_list):
                        rb = rpool.tile([64, 512], F32, tag="rb", name=f"rb{i}_{g}")
                        nc.sync.dma_start(
                            rb[:],
                            rcd[i, g]
                            .rearrange("(a n) -> a n", a=1)
                            .to_broadcast((64, 512)),
                        )
                        nc.vector.tensor_mul(ots[0:64, :], ots[0:64, :], rb[:])
                        nc.sync.dma_start(
                            outd[i][:, g * 512 : (g + 1) * 512], ots[0:64, :]
                        )
                        yield
                    state[i]["ots"] = []

            fillers = deque()

            def pump(n):
                while n > 0 and fillers:
                    try:
                        next(fillers[0])
                        n -= 1
                    except StopIteration:
                        fillers.popleft()

            def drain(gen=None):
                while fillers and (gen is None or gen in fillers):
                    pump(1)

            def unit(i, jb):
                for kc in range(NKC // 2):
                    sc_pair(i, jb, kc)
                    pump(PUMPS_PER_PAIR)

            # head 0 QKV runs eagerly; afterwards QKV(i+1) + AV trail the
            # score stream as interleaved filler, lagging by one q-block
            g0 = qkv_steps(0)
            fillers.append(g0)
            drain(g0)
            unit(0, 0)
            for i in range(HPC):
                if i > 0:
                    fillers.append(av_steps(i - 1, 1))
                    unit(i, 0)
                fillers.append(av_steps(i, 0))
                if i + 1 < HPC:
                    g = qkv_steps(i + 1)
                    fillers.append(g)
                    unit(i, 1)
                    drain(g)
                else:
                    unit(i, 1)
            fillers.append(av_steps(HPC - 1, 1))
            drain()

    _split_multi_waits(nc)
    _BUILT = nc
    return nc


def _core_inputs(sequences, wq, bq, wk, bk, wv, bv):
    import ml_dtypes

    bf16 = ml_dtypes.bfloat16
    xh = np.asarray(sequences, dtype=np.float32).reshape(B, S, H, DH)
    wq, bq = np.asarray(wq, np.float32), np.asarray(bq, np.float32)
    wk, bk = np.asarray(wk, np.float32), np.asarray(bk, np.float32)
    wv, bv = np.asarray(wv, np.float32), np.asarray(bv, np.float32)
    in_maps = []
    for c in range(NCORES):
        xT = np.empty((HPC, 128, S), dtype=bf16)
        wqk = np.empty((HPC, 64, 2, 64), dtype=bf16)
        wvT = np.empty((HPC, 128, 64), dtype=bf16)
        bqk = np.empty((HPC, 128, 2), dtype=np.float32)
        bvv = np.empty((HPC, 1, 64), dtype=np.float32)
        for i in range(HPC):
            f = c * HPC + i
            b, h = f // H, f % H
            xt = np.ascontiguousarray(xh[b, :, h, :].T).astype(bf16)
            xT[i, 0:64] = xt
            xT[i, 64:128] = xt
            wqk[i, :, 0, :] = wq[h].T.astype(bf16)
            wqk[i, :, 1, :] = wk[h].T.astype(bf16)
            wvT[i, 0:64] = wv[h].T.astype(bf16)
            wvT[i, 64:128] = wv[h].T.astype(bf16)
            bqk[i, 0:64, 0] = bq[h]
            bqk[i, 64:128, 0] = bq[h]
            bqk[i, 0:64, 1] = bk[h]
            bqk[i, 64:128, 1] = bk[h]
            bvv[i, 0] = bv[h]
        in_maps.append({"xT": xT, "wqk": wqk, "wvT": wvT, "bqk": bqk, "bv": bvv})
    return in_maps


def _gather(results):
    out = np.empty((B, S, H, DH), np.float32)
    for c in range(NCORES):
        o = np.asarray(results[c]["out"])  # [HPC, 64, S]
        for i in range(HPC):
            f = c * HPC + i
            b, h = f // H, f % H
            out[b, :, h, :] = o[i].T
    return out.reshape(B, S, D)


def kernel(sequences, wq, bq, wk, bk, wv, bv):
    from concourse.bass_utils import run_bass_kernel_spmd

    nc = build()
    in_maps = _core_inputs(sequences, wq, bq, wk, bk, wv, bv)
    res = run_bass_kernel_spmd(nc, in_maps, list(range(NCORES)))
    return _gather(res.results)



# revision 18
# speedup vs baseline: 1.3721x; 1.3721x over previous
"""Multi-head attention (B=4, S=2048, D=768, H=12) on 8 Trainium2 cores.

Sharding: the 48 (batch, head) pairs are data-parallel; each core gets 6.

Math restructure (exact):
  scores = (XWq^T+bq)(XWk^T+bk)^T -> softmax-invariant terms dropped:
    s_ij = x_i Wqk x_j^T + c_j   with Wqk = Wq^T Wk, c = X (bq Wk)^T
  The per-k-column bias c_j rides into the exp for free (per-partition
  bias port on ACT, add-constant on DVE), so K^T is the raw input and
  only one projection (qhat = X Wqk) is computed on device.
  V bias folds into a rank-1 (K=1) accumulating matmul.
  Normalization happens on the HOST: the kernel ships numerator rows
  plus a denominator row ([65, S] per head) produced by an appended
  ones-column in V_aug.

Engine split (the softmax exp is the throughput wall):
  per score pair, tile A -> ACT native exp (PSUM->SBUF bf16),
  tile B -> DVE Schraudolph: i16 = s*(A/8) + (B + A*c/8), bitcast bf16.
  (max|s/8| ~ 11, so i16 stays in [14k, 19k]: no overflow, no sign flip.)

Precision: fp16 x/qhat for the score matmuls (cuts bf16 rounding 8x),
bf16 P and V_aug, fp32 PSUM everywhere; host-measured rel err ~9.6e-3
at a 50/50 ACT/DVE split (gate is 2e-2).

PSUM: mm pool 3x[128,1024]f32 (score pairs / qhat / V) + av pool
2x[65,512]f32 = exactly 8 banks.
"""

import sys
from collections import deque

for _p in ("/opt/trn_rl_repo",):
    if _p not in sys.path:
        sys.path.insert(0, _p)

import numpy as np

B, S, D, H = 4, 2048, 768, 12
DH = 64
NCORES = 8
HPC = (B * H) // NCORES  # 6 heads per core
NKC = S // 128  # 16 k-chunks
NQB = 2  # q blocks of 1024
QB = S // NQB
PUMPS_PER_PAIR = 3

TRICK_A = 128.0 / np.log(2.0)  # 184.664965
TRICK_B = 127.0 * 128.0 - 5.57  # bf16 exponent bias, sawtooth-centered


def _split_multi_waits(nc):
    """This walrus build rejects >1 sync wait per instruction. Insert
    single-wait NoOps (same engine, so same instruction stream) ahead of
    any instruction carrying several waits."""
    import bass_rust
    import concourse.mybir as mybir

    n_split = 0
    for f in nc.m.functions:
        for bb in f.blocks:
            out = []
            dirty = False
            for inst in bb.instructions:
                si = inst.sync_info
                if si is not None and len(si.on_wait) > 1:
                    waits = list(si.on_wait)
                    for j, w in enumerate(waits[:-1]):
                        nop = mybir.InstNoOp(name=f"{inst.name}-w{j}", ins=[], outs=[])
                        nop.engine = inst.engine
                        nop.sync_info = bass_rust.SyncInfo(on_wait=[w], on_update=[])
                        out.append(nop)
                    si.on_wait = waits[-1:]
                    dirty = True
                    n_split += 1
                out.append(inst)
            if dirty:
                bb.instructions = out
    return n_split


_BUILT = None


def build():
    global _BUILT
    if _BUILT is not None:
        return _BUILT
    import concourse.bass as bass
    import concourse.mybir as mybir
    import concourse.tile as tile

    F32 = mybir.dt.float32
    F16 = mybir.dt.float16
    BF = mybir.dt.bfloat16
    I16 = mybir.dt.int16
    AF = mybir.ActivationFunctionType
    ALU = mybir.AluOpType

    nc = bass.Bass()
    xTd = nc.dram_tensor("xT", [HPC, 128, S], F16, kind="ExternalInput")
    wqkd = nc.dram_tensor("wqk", [HPC, 64, 64], F16, kind="ExternalInput")
    wvTd = nc.dram_tensor("wvT", [HPC, 128, 64], F16, kind="ExternalInput")
    cbd = nc.dram_tensor("cb", [HPC, 128, NKC], F32, kind="ExternalInput")
    tbd = nc.dram_tensor("tb", [HPC, 128, NKC], F32, kind="ExternalInput")
    outd = nc.dram_tensor("out", [HPC, 65, S], F32, kind="ExternalOutput")

    with tile.TileContext(nc) as tc:
        with (
            tc.tile_pool(name="x", bufs=2) as xpool,
            tc.tile_pool(name="w", bufs=2) as wpool,
            tc.tile_pool(name="qh", bufs=2) as qpool,
            tc.tile_pool(name="v", bufs=2) as vpool,
            tc.tile_pool(name="pt", bufs=2 * NKC) as ptpool,
            tc.tile_pool(name="ot", bufs=3) as otpool,
            tc.tile_pool(name="mm", bufs=3, space="PSUM") as mmpool,
            tc.tile_pool(name="av", bufs=2, space="PSUM") as avpool,
        ):
            state = {}

            def qkv_steps(i):
                x_t = xpool.tile([128, S], F16, tag="x", name=f"x{i}")
                nc.gpsimd.dma_start(x_t[:], xTd[i])
                wqk_t = wpool.tile([64, 64], F16, tag="wqk", name=f"wqk{i}")
                nc.gpsimd.dma_start(wqk_t[:], wqkd[i])
                wv_t = wpool.tile([128, 64], F16, tag="wv", name=f"wv{i}")
                nc.gpsimd.dma_start(wv_t[:], wvTd[i])
                cb_t = wpool.tile([128, NKC], F32, tag="cb", name=f"cb{i}")
                nc.gpsimd.dma_start(cb_t[:], cbd[i])
                tb_t = wpool.tile([128, NKC], F32, tag="tb", name=f"tb{i}")
                nc.gpsimd.dma_start(tb_t[:], tbd[i])
                yield

                # qhat^T = Wqk^T X^T, duplicated into both partition halves
                qhat = qpool.tile([128, S], F16, tag="qh", name=f"qh{i}")
                for qm in range(NQB):
                    ps = mmpool.tile([128, QB], F32, tag="mm", name=f"qp{i}_{qm}")
                    sl = slice(qm * QB, (qm + 1) * QB)
                    for hh in range(2):
                        q0 = qm * QB + hh * 512
                        rhs = x_t[0:64, q0 : q0 + 512]
                        psl = slice(hh * 512, (hh + 1) * 512)
                        nc.tensor.matmul(
                            ps[0:64, psl], wqk_t[:], rhs, tile_position=(0, 0)
                        )
                        nc.tensor.matmul(
                            ps[64:128, psl], wqk_t[:], rhs, tile_position=(0, 64)
                        )
                    nc.scalar.activation(qhat[:, sl], ps[:], AF.Copy)
                    yield

                # V_aug: [k-in-chunk, chunk, e + ones]; V bias added on host
                vhat = vpool.tile([128, NKC, 65], BF, tag="v", name=f"v{i}")
                nc.vector.memset(vhat[:, :, 64:65], 1.0)
                vps = mmpool.tile([128, NKC * 64], F32, tag="mm", name=f"vp{i}")
                for p in range(8):
                    nc.tensor.matmul(
                        vps[:, p * 64 : (p + 1) * 64],
                        x_t[0:64, p * 128 : (p + 1) * 128],
                        wv_t[0:64, :],
                        tile_position=(0, 0),
                    )
                    nc.tensor.matmul(
                        vps[:, (p + 8) * 64 : (p + 9) * 64],
                        x_t[64:128, (p + 8) * 128 : (p + 9) * 128],
                        wv_t[64:128, :],
                        tile_position=(64, 0),
                    )
                    if p % 2 == 1:
                        yield
                nc.vector.tensor_copy(
                    vhat[:, :, 0:64],
                    vps[:].rearrange("p (c e) -> p c e", e=64),
                )
                yield
                state[i] = {
                    "x": x_t,
                    "qh": qhat,
                    "v": vhat,
                    "cb": cb_t,
                    "tb": tb_t,
                    "pt": {},
                }

            def sc_pair(i, jb, p):
                """k-chunk pair (p, p+8) scores + exps for q block jb.
                Chunk p -> ACT native exp; chunk p+8 -> DVE bit-trick."""
                st = state[i]
                x_t, qhat = st["x"], st["qh"]
                pt = st["pt"].setdefault(jb, [None] * NKC)
                qsl = slice(jb * QB, (jb + 1) * QB)
                tA = mmpool.tile([128, QB], F32, tag="mm", name=f"sA{i}_{jb}_{p}")
                lA = x_t[0:64, p * 128 : (p + 1) * 128]
                for hh in range(2):
                    q0 = jb * QB + hh * 512
                    nc.tensor.matmul(
                        tA[:, hh * 512 : (hh + 1) * 512],
                        lA,
                        qhat[0:64, q0 : q0 + 512],
                        tile_position=(0, 0),
                    )
                tB = mmpool.tile([128, QB], F32, tag="mm", name=f"sB{i}_{jb}_{p}")
                lB = x_t[64:128, (p + 8) * 128 : (p + 9) * 128]
                for hh in range(2):
                    q0 = jb * QB + hh * 512
                    nc.tensor.matmul(
                        tB[:, hh * 512 : (hh + 1) * 512],
                        lB,
                        qhat[64:128, q0 : q0 + 512],
                        tile_position=(64, 0),
                    )
                pA = ptpool.tile([128, QB], BF, tag="pt", name=f"pA{i}_{jb}_{p}")
                nc.scalar.activation(
                    pA[:], tA[:], AF.Exp, bias=st["cb"][:, p : p + 1], scale=0.125
                )
                pB = ptpool.tile([128, QB], I16, tag="pt", name=f"pB{i}_{jb}_{p}")
                nc.vector.tensor_scalar(
                    pB[:],
                    tB[:],
                    TRICK_A / 8.0,
                    st["tb"][:, p + 8 : p + 9],
                    ALU.mult,
                    ALU.add,
                )
                pt[p] = pA[:]
                pt[p + 8] = pB[:].bitcast(BF)

            def av_steps(i, jb):
                """AV numerator + denominator row; output DMA'd unnormalized
                ([65, S] fp32); the host divides."""
                st = state[i]
                vhat = st["v"]
                pt = st["pt"].pop(jb)
                for qm in range(QB // 512):
                    av = avpool.tile([65, 512], F32, tag="av", name=f"av{i}_{jb}_{qm}")
                    for kc in range(NKC):
                        nc.tensor.matmul(
                            av[:],
                            vhat[:, kc, :],
                            pt[kc][:, qm * 512 : (qm + 1) * 512],
                            start=(kc == 0),
                            stop=(kc == NKC - 1),
                        )
                        if kc % 4 == 3:
                            yield
                    ots = otpool.tile([65, 512], F32, tag="ot", name=f"ot{i}_{jb}_{qm}")
                    nc.vector.tensor_copy(ots[:], av[:])
                    nc.sync.dma_start(
                        outd[i][:, jb * QB + qm * 512 : jb * QB + (qm + 1) * 512],
                        ots[:],
                    )
                    yield

            fillers = deque()

            def pump(n):
                while n > 0 and fillers:
                    try:
                        next(fillers[0])
                        n -= 1
                    except StopIteration:
                        fillers.popleft()

            def drain(gen=None):
                while fillers and (gen is None or gen in fillers):
                    pump(1)

            def unit(i, jb):
                for p in range(NKC // 2):
                    sc_pair(i, jb, p)
                    pump(PUMPS_PER_PAIR)

            g0 = qkv_steps(0)
            fillers.append(g0)
            drain(g0)
            unit(0, 0)
            for i in range(HPC):
                if i > 0:
                    fillers.append(av_steps(i - 1, 1))
                    unit(i, 0)
                fillers.append(av_steps(i, 0))
                if i + 1 < HPC:
                    g = qkv_steps(i + 1)
                    fillers.append(g)
                    unit(i, 1)
                    drain(g)
                else:
                    unit(i, 1)
            fillers.append(av_steps(HPC - 1, 1))
            drain()

    _split_multi_waits(nc)
    _BUILT = nc
    return nc


def _core_inputs(sequences, wq, bq, wk, bk, wv, bv):
    f16 = np.float16
    xh = np.asarray(sequences, dtype=np.float32).reshape(B, S, H, DH)
    wq = np.asarray(wq, np.float32)
    bq = np.asarray(bq, np.float32)
    wk = np.asarray(wk, np.float32)
    wv = np.asarray(wv, np.float32)
    bv = np.asarray(bv, np.float32)
    in_maps = []
    for c in range(NCORES):
        xT = np.empty((HPC, 128, S), dtype=f16)
        wqk = np.empty((HPC, 64, 64), dtype=f16)
        wvT = np.empty((HPC, 128, 64), dtype=f16)
        cb = np.empty((HPC, 128, NKC), dtype=np.float32)
        tb = np.empty((HPC, 128, NKC), dtype=np.float32)
        for i in range(HPC):
            f = c * HPC + i
            b, h = f // H, f % H
            xbh = xh[b, :, h, :]  # [S, 64]
            xt = np.ascontiguousarray(xbh.T).astype(f16)
            xT[i, 0:64] = xt
            xT[i, 64:128] = xt
            wqk[i] = (wq[h].T @ wk[h]).astype(f16)
            wvT[i, 0:64] = wv[h].T.astype(f16)
            wvT[i, 64:128] = wv[h].T.astype(f16)
            btil = bq[h] @ wk[h]  # [64]
            c8 = (xbh @ btil) / 8.0  # [S]
            cb[i] = c8.reshape(NKC, 128).T
            tb[i] = TRICK_B + TRICK_A * cb[i]
        in_maps.append({"xT": xT, "wqk": wqk, "wvT": wvT, "cb": cb, "tb": tb})
    return in_maps


def _gather(results, bv):
    bv = np.asarray(bv, np.float32)
    out = np.empty((B, S, H, DH), np.float32)
    for c in range(NCORES):
        o = np.asarray(results[c]["out"])  # [HPC, 65, S]
        for i in range(HPC):
            f = c * HPC + i
            b, h = f // H, f % H
            out[b, :, h, :] = (o[i, 0:64] / o[i, 64:65]).T + bv[h][None, :]
    return out.reshape(B, S, D)


def kernel(sequences, wq, bq, wk, bk, wv, bv):
    from concourse.bass_utils import run_bass_kernel_spmd

    nc = build()
    in_maps = _core_inputs(sequences, wq, bq, wk, bk, wv, bv)
    res = run_bass_kernel_spmd(nc, in_maps, list(range(NCORES)))
    return _gather(res.results, bv)


# revision 22
# speedup vs baseline: 1.4204x; 1.0352x over previous
"""Multi-head attention (B=4, S=2048, D=768, H=12) on 8 Trainium2 cores.

Sharding: the 48 (batch, head) pairs are data-parallel; each core gets 6.

Math restructure (exact):
  scores = (XWq^T+bq)(XWk^T+bk)^T -> softmax-invariant terms dropped:
    s_ij = x_i Wqk x_j^T + c_j   with Wqk = Wq^T Wk, c = X (bq Wk)^T
  The per-k-column bias c_j rides into the exp for free (per-partition
  bias port on ACT, add-constant on DVE), so K^T is the raw input and
  only one projection (qhat = X Wqk) is computed on device.
  V bias folds into a rank-1 (K=1) accumulating matmul.
  Normalization happens on the HOST: the kernel ships numerator rows
  plus a denominator row ([65, S] per head) produced by an appended
  ones-column in V_aug.

Engine split (the softmax exp is the throughput wall):
  per score pair, tile A -> ACT native exp (PSUM->SBUF bf16),
  tile B -> DVE Schraudolph: i16 = s*(A/8) + (B + A*c/8), bitcast bf16.
  (max|s/8| ~ 11, so i16 stays in [14k, 19k]: no overflow, no sign flip.)

Precision: fp16 x/qhat for the score matmuls (cuts bf16 rounding 8x),
bf16 P and V_aug, fp32 PSUM everywhere; host-measured rel err ~9.6e-3
at a 50/50 ACT/DVE split (gate is 2e-2).

PSUM: mm pool 3x[128,1024]f32 (score pairs / qhat / V) + av pool
2x[65,512]f32 = exactly 8 banks.
"""

import sys
from collections import deque

for _p in ("/opt/trn_rl_repo",):
    if _p not in sys.path:
        sys.path.insert(0, _p)

import numpy as np

B, S, D, H = 4, 2048, 768, 12
DH = 64
NCORES = 8
HPC = (B * H) // NCORES  # 6 heads per core
NKC = S // 128  # 16 k-chunks
NQB = 2  # q blocks of 1024
QB = S // NQB
PUMPS_PER_PAIR = 3

TRICK_A = 128.0 / np.log(2.0)  # 184.664965
TRICK_B = 127.0 * 128.0 - 5.57  # bf16 exponent bias, sawtooth-centered


def _split_multi_waits(nc):
    """This walrus build rejects >1 sync wait per instruction. Insert
    single-wait NoOps (same engine, so same instruction stream) ahead of
    any instruction carrying several waits."""
    import bass_rust
    import concourse.mybir as mybir

    n_split = 0
    for f in nc.m.functions:
        for bb in f.blocks:
            out = []
            dirty = False
            for inst in bb.instructions:
                si = inst.sync_info
                if si is not None and len(si.on_wait) > 1:
                    waits = list(si.on_wait)
                    for j, w in enumerate(waits[:-1]):
                        nop = mybir.InstNoOp(name=f"{inst.name}-w{j}", ins=[], outs=[])
                        nop.engine = inst.engine
                        nop.sync_info = bass_rust.SyncInfo(on_wait=[w], on_update=[])
                        out.append(nop)
                    si.on_wait = waits[-1:]
                    dirty = True
                    n_split += 1
                out.append(inst)
            if dirty:
                bb.instructions = out
    return n_split


_BUILT = None


def build():
    global _BUILT
    if _BUILT is not None:
        return _BUILT
    import concourse.bass as bass
    import concourse.mybir as mybir
    import concourse.tile as tile

    F32 = mybir.dt.float32
    F16 = mybir.dt.float16
    BF = mybir.dt.bfloat16
    I16 = mybir.dt.int16
    AF = mybir.ActivationFunctionType
    ALU = mybir.AluOpType

    nc = bass.Bass()
    xTd = nc.dram_tensor("xT", [HPC, 128, S], F16, kind="ExternalInput")
    wqkd = nc.dram_tensor("wqk", [HPC, 64, 64], F16, kind="ExternalInput")
    wvTd = nc.dram_tensor("wvT", [HPC, 128, 64], F16, kind="ExternalInput")
    cbd = nc.dram_tensor("cb", [HPC, 128, NKC], F32, kind="ExternalInput")
    tbd = nc.dram_tensor("tb", [HPC, 128, NKC], F32, kind="ExternalInput")
    outd = nc.dram_tensor("out", [HPC, 65, S], F32, kind="ExternalOutput")

    with tile.TileContext(nc) as tc:
        with (
            tc.tile_pool(name="x", bufs=2) as xpool,
            tc.tile_pool(name="w", bufs=2) as wpool,
            tc.tile_pool(name="qh", bufs=2) as qpool,
            tc.tile_pool(name="v", bufs=2) as vpool,
            tc.tile_pool(name="pt", bufs=2 * NKC) as ptpool,
            tc.tile_pool(name="ot", bufs=2) as otpool,
            tc.tile_pool(name="mm", bufs=3, space="PSUM") as mmpool,
            tc.tile_pool(name="av", bufs=2, space="PSUM") as avpool,
        ):
            state = {}

            def qkv_steps(i):
                x_t = xpool.tile([128, S], F16, tag="x", name=f"x{i}")
                nc.gpsimd.dma_start(x_t[:], xTd[i])
                wqk_t = wpool.tile([64, 64], F16, tag="wqk", name=f"wqk{i}")
                nc.gpsimd.dma_start(wqk_t[:], wqkd[i])
                wv_t = wpool.tile([128, 64], F16, tag="wv", name=f"wv{i}")
                nc.gpsimd.dma_start(wv_t[:], wvTd[i])
                cb_t = wpool.tile([128, NKC], F32, tag="cb", name=f"cb{i}")
                nc.gpsimd.dma_start(cb_t[:], cbd[i])
                tb_t = wpool.tile([128, NKC], F32, tag="tb", name=f"tb{i}")
                nc.gpsimd.dma_start(tb_t[:], tbd[i])
                yield

                # qhat^T = Wqk^T X^T, duplicated into both partition halves
                qhat = qpool.tile([128, S], F16, tag="qh", name=f"qh{i}")
                for qm in range(NQB):
                    ps = mmpool.tile([128, QB], F32, tag="mm", name=f"qp{i}_{qm}")
                    sl = slice(qm * QB, (qm + 1) * QB)
                    for hh in range(2):
                        q0 = qm * QB + hh * 512
                        rhs = x_t[0:64, q0 : q0 + 512]
                        psl = slice(hh * 512, (hh + 1) * 512)
                        nc.tensor.matmul(
                            ps[0:64, psl], wqk_t[:], rhs, tile_position=(0, 0)
                        )
                        nc.tensor.matmul(
                            ps[64:128, psl], wqk_t[:], rhs, tile_position=(0, 64)
                        )
                    nc.scalar.activation(qhat[:, sl], ps[:], AF.Copy)
                    yield

                # V_aug: [k-in-chunk, chunk, e + ones]; V bias added on host
                vhat = vpool.tile([128, NKC, 65], BF, tag="v", name=f"v{i}")
                nc.vector.memset(vhat[:, :, 64:65], 1.0)
                vps = mmpool.tile([128, NKC * 64], F32, tag="mm", name=f"vp{i}")
                for p in range(8):
                    nc.tensor.matmul(
                        vps[:, p * 64 : (p + 1) * 64],
                        x_t[0:64, p * 128 : (p + 1) * 128],
                        wv_t[0:64, :],
                        tile_position=(0, 0),
                    )
                    nc.tensor.matmul(
                        vps[:, (p + 8) * 64 : (p + 9) * 64],
                        x_t[64:128, (p + 8) * 128 : (p + 9) * 128],
                        wv_t[64:128, :],
                        tile_position=(64, 0),
                    )
                    if p % 2 == 1:
                        yield
                nc.vector.tensor_copy(
                    vhat[:, :, 0:64],
                    vps[:].rearrange("p (c e) -> p c e", e=64),
                )
                yield
                state[i] = {
                    "x": x_t,
                    "qh": qhat,
                    "v": vhat,
                    "cb": cb_t,
                    "tb": tb_t,
                    "pt": {},
                }

            def sc_pair(i, jb, p):
                """k-chunk pair (p, p+8) scores + exps for q block jb.
                Chunk p -> ACT native exp; chunk p+8 -> DVE bit-trick."""
                st = state[i]
                x_t, qhat = st["x"], st["qh"]
                pt = st["pt"].setdefault(jb, [None] * NKC)
                qsl = slice(jb * QB, (jb + 1) * QB)
                tA = mmpool.tile([128, QB], F32, tag="mm", name=f"sA{i}_{jb}_{p}")
                lA = x_t[0:64, p * 128 : (p + 1) * 128]
                for hh in range(2):
                    q0 = jb * QB + hh * 512
                    nc.tensor.matmul(
                        tA[:, hh * 512 : (hh + 1) * 512],
                        lA,
                        qhat[0:64, q0 : q0 + 512],
                        tile_position=(0, 0),
                    )
                tB = mmpool.tile([128, QB], F32, tag="mm", name=f"sB{i}_{jb}_{p}")
                lB = x_t[64:128, (p + 8) * 128 : (p + 9) * 128]
                for hh in range(2):
                    q0 = jb * QB + hh * 512
                    nc.tensor.matmul(
                        tB[:, hh * 512 : (hh + 1) * 512],
                        lB,
                        qhat[64:128, q0 : q0 + 512],
                        tile_position=(64, 0),
                    )
                pA = ptpool.tile([128, QB], BF, tag="pt", name=f"pA{i}_{jb}_{p}")
                nc.scalar.activation(
                    pA[:], tA[:], AF.Exp, bias=st["cb"][:, p : p + 1], scale=0.125
                )
                pB = ptpool.tile([128, QB], I16, tag="pt", name=f"pB{i}_{jb}_{p}")
                nc.vector.tensor_scalar(
                    pB[:],
                    tB[:],
                    TRICK_A / 8.0,
                    st["tb"][:, p + 8 : p + 9],
                    ALU.mult,
                    ALU.add,
                )
                pt[p] = pA[:]
                pt[p + 8] = pB[:].bitcast(BF)

            def av_steps(i, jb):
                """AV numerator + denominator row; output DMA'd unnormalized
                ([65, S] fp32); the host divides."""
                st = state[i]
                vhat = st["v"]
                pt = st["pt"].pop(jb)
                avs = [
                    avpool.tile([65, 512], F32, tag="av", name=f"av{i}_{jb}_{qm}")
                    for qm in range(QB // 512)
                ]
                for kc in range(NKC):
                    for qm in range(QB // 512):
                        nc.tensor.matmul(
                            avs[qm][:],
                            vhat[:, kc, :],
                            pt[kc][:, qm * 512 : (qm + 1) * 512],
                            start=(kc == 0),
                            stop=(kc == NKC - 1),
                        )
                    if kc % 2 == 1:
                        yield
                for qm in range(QB // 512):
                    ots = otpool.tile(
                        [65, 512], F32, tag="ot", name=f"ot{i}_{jb}_{qm}"
                    )
                    nc.scalar.activation(ots[:], avs[qm][:], AF.Copy)
                    nc.sync.dma_start(
                        outd[i][:, jb * QB + qm * 512 : jb * QB + (qm + 1) * 512],
                        ots[:],
                    )
                    yield

            fillers = deque()

            def pump(n):
                while n > 0 and fillers:
                    try:
                        next(fillers[0])
                        n -= 1
                    except StopIteration:
                        fillers.popleft()

            def drain(gen=None):
                while fillers and (gen is None or gen in fillers):
                    pump(1)

            def unit(i, jb):
                for p in range(NKC // 2):
                    sc_pair(i, jb, p)
                    pump(PUMPS_PER_PAIR)

            g0 = qkv_steps(0)
            fillers.append(g0)
            drain(g0)
            unit(0, 0)
            for i in range(HPC):
                if i > 0:
                    fillers.append(av_steps(i - 1, 1))
                    unit(i, 0)
                fillers.append(av_steps(i, 0))
                if i + 1 < HPC:
                    g = qkv_steps(i + 1)
                    fillers.append(g)
                    unit(i, 1)
                    drain(g)
                else:
                    unit(i, 1)
            fillers.append(av_steps(HPC - 1, 1))
            drain()

    _split_multi_waits(nc)
    _BUILT = nc
    return nc


def _core_inputs(sequences, wq, bq, wk, bk, wv, bv):
    f16 = np.float16
    xh = np.asarray(sequences, dtype=np.float32).reshape(B, S, H, DH)
    wq = np.asarray(wq, np.float32)
    bq = np.asarray(bq, np.float32)
    wk = np.asarray(wk, np.float32)
    wv = np.asarray(wv, np.float32)
    bv = np.asarray(bv, np.float32)
    in_maps = []
    for c in range(NCORES):
        xT = np.empty((HPC, 128, S), dtype=f16)
        wqk = np.empty((HPC, 64, 64), dtype=f16)
        wvT = np.empty((HPC, 128, 64), dtype=f16)
        cb = np.empty((HPC, 128, NKC), dtype=np.float32)
        tb = np.empty((HPC, 128, NKC), dtype=np.float32)
        for i in range(HPC):
            f = c * HPC + i
            b, h = f // H, f % H
            xbh = xh[b, :, h, :]  # [S, 64]
            xt = np.ascontiguousarray(xbh.T).astype(f16)
            xT[i, 0:64] = xt
            xT[i, 64:128] = xt
            wqk[i] = (wq[h].T @ wk[h]).astype(f16)
            wvT[i, 0:64] = wv[h].T.astype(f16)
            wvT[i, 64:128] = wv[h].T.astype(f16)
            btil = bq[h] @ wk[h]  # [64]
            c8 = (xbh @ btil) / 8.0  # [S]
            cb[i] = c8.reshape(NKC, 128).T
            tb[i] = TRICK_B + TRICK_A * cb[i]
        in_maps.append({"xT": xT, "wqk": wqk, "wvT": wvT, "cb": cb, "tb": tb})
    return in_maps


def _gather(results, bv):
    bv = np.asarray(bv, np.float32)
    out = np.empty((B, S, H, DH), np.float32)
    for c in range(NCORES):
        o = np.asarray(results[c]["out"])  # [HPC, 65, S]
        for i in range(HPC):
            f = c * HPC + i
            b, h = f // H, f % H
            out[b, :, h, :] = (o[i, 0:64] / o[i, 64:65]).T + bv[h][None, :]
    return out.reshape(B, S, D)


def kernel(sequences, wq, bq, wk, bk, wv, bv):
    from concourse.bass_utils import run_bass_kernel_spmd

    nc = build()
    in_maps = _core_inputs(sequences, wq, bq, wk, bk, wv, bv)
    res = run_bass_kernel_spmd(nc, in_maps, list(range(NCORES)))
    return _gather(res.results, bv)


# revision 26
# speedup vs baseline: 1.4591x; 1.0273x over previous
"""Multi-head attention (B=4, S=2048, D=768, H=12) on 8 Trainium2 cores.

Sharding: the 48 (batch, head) pairs are data-parallel; each core gets 6.

Math restructure (exact):
  scores = (XWq^T+bq)(XWk^T+bk)^T -> softmax-invariant terms dropped:
    s_ij = x_i Wqk x_j^T + c_j   with Wqk = Wq^T Wk, c = X (bq Wk)^T
  The per-k-column bias c_j rides into the exp for free (per-partition
  bias port on ACT, add-constant on DVE), so K^T is the raw input and
  only one projection (qhat = X Wqk) is computed on device.
  V bias folds into a rank-1 (K=1) accumulating matmul.
  Normalization happens on the HOST: the kernel ships numerator rows
  plus a denominator row ([65, S] per head) produced by an appended
  ones-column in V_aug.

Engine split (the softmax exp is the throughput wall):
  per score pair, tile A -> ACT native exp (PSUM->SBUF bf16),
  tile B -> DVE Schraudolph: i16 = s*(A/8) + (B + A*c/8), bitcast bf16.
  (max|s/8| ~ 11, so i16 stays in [14k, 19k]: no overflow, no sign flip.)

Precision: fp16 x/qhat for the score matmuls (cuts bf16 rounding 8x),
bf16 P and V_aug, fp32 PSUM everywhere; host-measured rel err ~9.6e-3
at a 50/50 ACT/DVE split (gate is 2e-2).

PSUM: mm pool 3x[128,1024]f32 (score pairs / qhat / V) + av pool
2x[65,512]f32 = exactly 8 banks.
"""

import sys
from collections import deque

for _p in ("/opt/trn_rl_repo",):
    if _p not in sys.path:
        sys.path.insert(0, _p)

import numpy as np

B, S, D, H = 4, 2048, 768, 12
DH = 64
NCORES = 8
HPC = (B * H) // NCORES  # 6 heads per core
NKC = S // 128  # 16 k-chunks
NQB = 2  # q blocks of 1024
QB = S // NQB
PUMPS_PER_PAIR = 2

TRICK_A = 128.0 / np.log(2.0)  # 184.664965
TRICK_B = 127.0 * 128.0 - 5.57  # bf16 exponent bias, sawtooth-centered


def _split_multi_waits(nc):
    """This walrus build rejects >1 sync wait per instruction. Insert
    single-wait NoOps (same engine, so same instruction stream) ahead of
    any instruction carrying several waits."""
    import bass_rust
    import concourse.mybir as mybir

    n_split = 0
    for f in nc.m.functions:
        for bb in f.blocks:
            out = []
            dirty = False
            for inst in bb.instructions:
                si = inst.sync_info
                if si is not None and len(si.on_wait) > 1:
                    waits = list(si.on_wait)
                    for j, w in enumerate(waits[:-1]):
                        nop = mybir.InstNoOp(name=f"{inst.name}-w{j}", ins=[], outs=[])
                        nop.engine = inst.engine
                        nop.sync_info = bass_rust.SyncInfo(on_wait=[w], on_update=[])
                        out.append(nop)
                    si.on_wait = waits[-1:]
                    dirty = True
                    n_split += 1
                out.append(inst)
            if dirty:
                bb.instructions = out
    return n_split


_BUILT = None


def build():
    global _BUILT
    if _BUILT is not None:
        return _BUILT
    import concourse.bass as bass
    import concourse.mybir as mybir
    import concourse.tile as tile

    F32 = mybir.dt.float32
    F16 = mybir.dt.float16
    BF = mybir.dt.bfloat16
    I16 = mybir.dt.int16
    AF = mybir.ActivationFunctionType
    ALU = mybir.AluOpType

    nc = bass.Bass()
    xTd = nc.dram_tensor("xT", [HPC, 128, S], F16, kind="ExternalInput")
    wqkd = nc.dram_tensor("wqk", [HPC, 64, 64], F16, kind="ExternalInput")
    wvTd = nc.dram_tensor("wvT", [HPC, 128, 64], F16, kind="ExternalInput")
    cbd = nc.dram_tensor("cb", [HPC, 128, NKC], F32, kind="ExternalInput")
    tbd = nc.dram_tensor("tb", [HPC, 128, NKC], F32, kind="ExternalInput")
    outd = nc.dram_tensor("out", [HPC, 65, S], F32, kind="ExternalOutput")

    with tile.TileContext(nc) as tc:
        with (
            tc.tile_pool(name="x", bufs=2) as xpool,
            tc.tile_pool(name="w", bufs=2) as wpool,
            tc.tile_pool(name="qh", bufs=2) as qpool,
            tc.tile_pool(name="v", bufs=2) as vpool,
            tc.tile_pool(name="pt", bufs=2 * NKC) as ptpool,
            tc.tile_pool(name="ot", bufs=2) as otpool,
            tc.tile_pool(name="mm", bufs=3, space="PSUM") as mmpool,
            tc.tile_pool(name="av", bufs=2, space="PSUM") as avpool,
        ):
            # warm the ACT exp table during the first DMAs
            warm = xpool.tile([1, 1], F32, tag="warm")
            nc.vector.memset(warm[:], 0.0)
            nc.scalar.activation(warm[:], warm[:], AF.Exp)

            state = {}

            def qkv_steps(i):
                x_t = xpool.tile([128, S], F16, tag="x", name=f"x{i}")
                nc.gpsimd.dma_start(x_t[:], xTd[i])
                wqk_t = wpool.tile([64, 64], F16, tag="wqk", name=f"wqk{i}")
                nc.sync.dma_start(wqk_t[:], wqkd[i])
                wv_t = wpool.tile([128, 64], F16, tag="wv", name=f"wv{i}")
                nc.sync.dma_start(wv_t[:], wvTd[i])
                cb_t = wpool.tile([128, NKC], F32, tag="cb", name=f"cb{i}")
                nc.sync.dma_start(cb_t[:], cbd[i])
                tb_t = wpool.tile([128, NKC], F32, tag="tb", name=f"tb{i}")
                nc.sync.dma_start(tb_t[:], tbd[i])
                yield

                # qhat^T = Wqk^T X^T, duplicated into both partition halves
                qhat = qpool.tile([128, S], F16, tag="qh", name=f"qh{i}")
                for qm in range(NQB):
                    ps = mmpool.tile([128, QB], F32, tag="mm", name=f"qp{i}_{qm}")
                    sl = slice(qm * QB, (qm + 1) * QB)
                    for hh in range(2):
                        q0 = qm * QB + hh * 512
                        rhs = x_t[0:64, q0 : q0 + 512]
                        psl = slice(hh * 512, (hh + 1) * 512)
                        nc.tensor.matmul(
                            ps[0:64, psl], wqk_t[:], rhs, tile_position=(0, 0)
                        )
                        nc.tensor.matmul(
                            ps[64:128, psl], wqk_t[:], rhs, tile_position=(0, 64)
                        )
                    nc.scalar.activation(qhat[:, sl], ps[:], AF.Copy)
                    yield

                # V_aug: [k-in-chunk, chunk, e + ones]; V bias added on host
                vhat = vpool.tile([128, NKC, 65], BF, tag="v", name=f"v{i}")
                nc.vector.memset(vhat[:, :, 64:65], 1.0)
                vps = mmpool.tile([128, NKC * 64], F32, tag="mm", name=f"vp{i}")
                for p in range(8):
                    nc.tensor.matmul(
                        vps[:, p * 64 : (p + 1) * 64],
                        x_t[0:64, p * 128 : (p + 1) * 128],
                        wv_t[0:64, :],
                        tile_position=(0, 0),
                    )
                    nc.tensor.matmul(
                        vps[:, (p + 8) * 64 : (p + 9) * 64],
                        x_t[64:128, (p + 8) * 128 : (p + 9) * 128],
                        wv_t[64:128, :],
                        tile_position=(64, 0),
                    )
                    if p % 2 == 1:
                        yield
                nc.vector.tensor_copy(
                    vhat[:, :, 0:64],
                    vps[:].rearrange("p (c e) -> p c e", e=64),
                )
                yield
                state[i] = {
                    "x": x_t,
                    "qh": qhat,
                    "v": vhat,
                    "cb": cb_t,
                    "tb": tb_t,
                    "pt": {},
                }

            def sc_pair(i, jb, p):
                """k-chunk pair (p, p+8) scores + exps for q block jb.
                Chunk p -> ACT native exp; chunk p+8 -> DVE bit-trick."""
                st = state[i]
                x_t, qhat = st["x"], st["qh"]
                pt = st["pt"].setdefault(jb, [None] * NKC)
                qsl = slice(jb * QB, (jb + 1) * QB)
                # interleave A/B halves: starts are pc-monotone, so issuing
                # A0,A1,B0,B1 would stall B0 behind A1's row-group wait
                tA = mmpool.tile([128, QB], F32, tag="mm", name=f"sA{i}_{jb}_{p}")
                tB = mmpool.tile([128, QB], F32, tag="mm", name=f"sB{i}_{jb}_{p}")
                lA = x_t[0:64, p * 128 : (p + 1) * 128]
                lB = x_t[64:128, (p + 8) * 128 : (p + 9) * 128]
                for hh in range(2):
                    q0 = jb * QB + hh * 512
                    sl = slice(hh * 512, (hh + 1) * 512)
                    nc.tensor.matmul(
                        tA[:, sl], lA, qhat[0:64, q0 : q0 + 512], tile_position=(0, 0)
                    )
                    nc.tensor.matmul(
                        tB[:, sl],
                        lB,
                        qhat[64:128, q0 : q0 + 512],
                        tile_position=(64, 0),
                    )
                pA = ptpool.tile([128, QB], BF, tag="pt", name=f"pA{i}_{jb}_{p}")
                nc.scalar.activation(
                    pA[:], tA[:], AF.Exp, bias=st["cb"][:, p : p + 1], scale=0.125
                )
                pB = ptpool.tile([128, QB], I16, tag="pt", name=f"pB{i}_{jb}_{p}")
                nc.vector.tensor_scalar(
                    pB[:],
                    tB[:],
                    TRICK_A / 8.0,
                    st["tb"][:, p + 8 : p + 9],
                    ALU.mult,
                    ALU.add,
                )
                pt[p] = pA[:]
                pt[p + 8] = pB[:].bitcast(BF)

            def av_steps(i, jb):
                """AV numerator + denominator row; output DMA'd unnormalized
                ([65, S] fp32); the host divides."""
                st = state[i]
                vhat = st["v"]
                pt = st["pt"].pop(jb)
                avs = [
                    avpool.tile([65, 512], F32, tag="av", name=f"av{i}_{jb}_{qm}")
                    for qm in range(QB // 512)
                ]
                for kc in range(NKC):
                    for qm in range(QB // 512):
                        nc.tensor.matmul(
                            avs[qm][:],
                            vhat[:, kc, :],
                            pt[kc][:, qm * 512 : (qm + 1) * 512],
                            start=(kc == 0),
                            stop=(kc == NKC - 1),
                        )
                    yield
                for qm in range(QB // 512):
                    ots = otpool.tile(
                        [65, 512], F32, tag="ot", name=f"ot{i}_{jb}_{qm}"
                    )
                    nc.scalar.activation(ots[:], avs[qm][:], AF.Copy)
                    nc.sync.dma_start(
                        outd[i][:, jb * QB + qm * 512 : jb * QB + (qm + 1) * 512],
                        ots[:],
                    )
                    yield

            fillers = deque()

            def pump(n):
                while n > 0 and fillers:
                    try:
                        next(fillers[0])
                        n -= 1
                    except StopIteration:
                        fillers.popleft()

            def drain(gen=None):
                while fillers and (gen is None or gen in fillers):
                    pump(1)

            def unit(i, jb):
                for p in range(NKC // 2):
                    sc_pair(i, jb, p)
                    pump(PUMPS_PER_PAIR)

            g0 = qkv_steps(0)
            fillers.append(g0)
            drain(g0)
            unit(0, 0)
            for i in range(HPC):
                if i > 0:
                    fillers.append(av_steps(i - 1, 1))
                    unit(i, 0)
                fillers.append(av_steps(i, 0))
                if i + 1 < HPC:
                    g = qkv_steps(i + 1)
                    fillers.append(g)
                    unit(i, 1)
                    drain(g)
                else:
                    unit(i, 1)
            fillers.append(av_steps(HPC - 1, 1))
            drain()

    _split_multi_waits(nc)
    _BUILT = nc
    return nc


def _core_inputs(sequences, wq, bq, wk, bk, wv, bv):
    f16 = np.float16
    xh = np.asarray(sequences, dtype=np.float32).reshape(B, S, H, DH)
    wq = np.asarray(wq, np.float32)
    bq = np.asarray(bq, np.float32)
    wk = np.asarray(wk, np.float32)
    wv = np.asarray(wv, np.float32)
    bv = np.asarray(bv, np.float32)
    in_maps = []
    for c in range(NCORES):
        xT = np.empty((HPC, 128, S), dtype=f16)
        wqk = np.empty((HPC, 64, 64), dtype=f16)
        wvT = np.empty((HPC, 128, 64), dtype=f16)
        cb = np.empty((HPC, 128, NKC), dtype=np.float32)
        tb = np.empty((HPC, 128, NKC), dtype=np.float32)
        for i in range(HPC):
            f = c * HPC + i
            b, h = f // H, f % H
            xbh = xh[b, :, h, :]  # [S, 64]
            xt = np.ascontiguousarray(xbh.T).astype(f16)
            xT[i, 0:64] = xt
            xT[i, 64:128] = xt
            wqk[i] = (wq[h].T @ wk[h]).astype(f16)
            wvT[i, 0:64] = wv[h].T.astype(f16)
            wvT[i, 64:128] = wv[h].T.astype(f16)
            btil = bq[h] @ wk[h]  # [64]
            c8 = (xbh @ btil) / 8.0  # [S]
            cb[i] = c8.reshape(NKC, 128).T
            tb[i] = TRICK_B + TRICK_A * cb[i]
        in_maps.append({"xT": xT, "wqk": wqk, "wvT": wvT, "cb": cb, "tb": tb})
    return in_maps


def _gather(results, bv):
    bv = np.asarray(bv, np.float32)
    out = np.empty((B, S, H, DH), np.float32)
    for c in range(NCORES):
        o = np.asarray(results[c]["out"])  # [HPC, 65, S]
        for i in range(HPC):
            f = c * HPC + i
            b, h = f // H, f % H
            out[b, :, h, :] = (o[i, 0:64] / o[i, 64:65]).T + bv[h][None, :]
    return out.reshape(B, S, D)


def kernel(sequences, wq, bq, wk, bk, wv, bv):
    from concourse.bass_utils import run_bass_kernel_spmd

    nc = build()
    in_maps = _core_inputs(sequences, wq, bq, wk, bk, wv, bv)
    res = run_bass_kernel_spmd(nc, in_maps, list(range(NCORES)))
    return _gather(res.results, bv)
